# revision 1
# baseline (speedup 1.0000x reference)
"""BinaryConv2d Trainium2 kernel (8-core batch-parallel, PE row-group packed).

Per image: top half rows on partitions 0:64, bottom half on 64:128 (each a
zero-padded bf16 slab of HALF+2 rows). Each 3x3 conv position runs as TWO
concurrent 64x64 matmuls on PE row groups (tile_position (0,0) and (64,0)),
accumulating into separate PSUM banks. Doubles PE utilization vs v1.
"""
import sys
import numpy as np
from contextlib import ExitStack

sys.path.insert(0, "/root/.axon_site/_ro/trn_rl_repo")
sys.path.insert(0, "/opt/trn_rl_repo")

import ml_dtypes
import concourse.bass as bass
import concourse.bacc as bacc
import concourse.mybir as mybir
import concourse.tile as tile
from concourse.bass_utils import run_bass_kernel_spmd

F32 = mybir.dt.float32
BF16 = mybir.dt.bfloat16

N_CORES = 8
B, CIN, COUT, KS = 32, 64, 64, 3
H = W = 160
B_CORE = B // N_CORES
HALF = H // 2          # rows per half
SH = HALF + 2          # slab rows per half (1 halo/pad row each side)
PW = W + 2
RPT = 3                # output rows per PSUM tile


def build_nc(n_img=B_CORE, h=H, w=W):
    half = h // 2
    sh = half + 2
    pw = w + 2
    nc = bacc.Bacc("TRN2", target_bir_lowering=False, debug=False, num_devices=N_CORES)
    x_in = nc.declare_dram_parameter("x", [n_img, CIN, h, w], F32, isOutput=False)
    wsgn_in = nc.declare_dram_parameter("wsgn", [128, 9 * 64], BF16, isOutput=False)
    scale_in = nc.declare_dram_parameter("scale", [64, 1], F32, isOutput=False)
    out_ext = nc.declare_dram_parameter("out", [n_img, COUT, h, w], F32, isOutput=True)

    n_tiles = (half + RPT - 1) // RPT

    with tile.TileContext(nc) as tc, ExitStack() as ctx:
        wpool = ctx.enter_context(tc.tile_pool(name="wpool", bufs=1))
        spool = ctx.enter_context(tc.tile_pool(name="spool", bufs=1))
        xpool = ctx.enter_context(tc.tile_pool(name="xpool", bufs=3))
        ppool = ctx.enter_context(tc.tile_pool(name="ppool", bufs=3, space="PSUM"))
        opool = ctx.enter_context(tc.tile_pool(name="opool", bufs=8))

        wt2 = wpool.tile([128, 9 * 64], BF16, name="wt2")
        nc.sync.dma_start(wt2[:], wsgn_in[:])
        sc = wpool.tile([64, 1], F32, name="sc")
        nc.sync.dma_start(sc[:], scale_in[:])

        # Two persistent slab buffers (manual ping-pong); pads zeroed once.
        slabs = []
        for i in range(2):
            slab = spool.tile([128, sh * pw], BF16, name=f"slab{i}", tag=f"slab{i}")
            s3 = slab.rearrange("p (r c) -> p r c", c=pw)
            # col pads: elements r*pw + {0, pw-1} for all slab rows
            nc.vector.memset(slab[:, 0 : (sh - 1) * pw + pw : pw], 0.0)
            nc.vector.memset(slab[:, pw - 1 : sh * pw : pw], 0.0)
            # row pads: top half row 0 (partitions 0:64), bottom half last row
            nc.vector.memset(s3[0:64, 0, :], 0.0)
            nc.vector.memset(s3[64:128, sh - 1, :], 0.0)
            slabs.append(slab)

        # staging: slab rows 1..half in chunks; leftovers done separately.
        # Finer chunks shorten the critical path to the image's first matmul.
        n_ch = 4 if half % 4 == 0 else (2 if half % 2 == 0 else 1)
        ch = half // n_ch  # slab rows per chunk (covers s=1..half)

        for img in range(n_img):
            slab = slabs[img % 2]
            s3 = slab.rearrange("p (r c) -> p r c", c=pw)

            # halo rows FIRST: bottom slab row 0 <- x row half-1 gates the very
            # first bottom-half tile, so it must not queue behind the big signs
            xs = xpool.tile([128, w], F32, name="xs", tag="xs")
            nc.sync.dma_start(xs[0:64, :], x_in[img, :, half : half + 1, :])
            nc.sync.dma_start(xs[64:128, :], x_in[img, :, half - 1 : half, :])
            nc.scalar.sign(s3[0:64, sh - 1, 1 : 1 + w], xs[0:64, :])
            nc.scalar.sign(s3[64:128, 0, 1 : 1 + w], xs[64:128, :])

            for c in range(n_ch):
                s_lo = 1 + c * ch  # slab row range [s_lo, s_lo+ch)
                xc = xpool.tile([128, ch * w], F32, name="xc", tag="xc")
                xc3 = xc.rearrange("p (r c) -> p r c", c=w)
                # top half: slab row s <- x row s-1
                nc.sync.dma_start(
                    xc[0:64, :], x_in[img, :, s_lo - 1 : s_lo - 1 + ch, :]
                )
                # bottom half: slab row s <- x row half - 1 + s
                nc.sync.dma_start(
                    xc[64:128, :],
                    x_in[img, :, half - 1 + s_lo : half - 1 + s_lo + ch, :],
                )
                nc.scalar.sign(s3[:, s_lo : s_lo + ch, 1 : 1 + w], xc3[:])

            for t in range(n_tiles):
                h0 = t * RPT
                R = min(RPT, half - h0)
                psumT = ppool.tile([64, R * w], F32, name="psumT", tag="psumT")
                psumB = ppool.tile([64, R * w], F32, name="psumB", tag="psumB")
                for kh in range(KS):
                    for kw in range(KS):
                        pos = kh * KS + kw
                        st, sp = (pos == 0), (pos == 8)
                        nc.tensor.matmul(
                            psumT[:],
                            wt2[0:64, pos * 64 : (pos + 1) * 64],
                            s3[0:64, h0 + kh : h0 + kh + R, kw : kw + w],
                            start=st, stop=sp,
                            tile_position=(0, 0),
                        )
                        nc.tensor.matmul(
                            psumB[:],
                            wt2[64:128, pos * 64 : (pos + 1) * 64],
                            s3[64:128, h0 + kh : h0 + kh + R, kw : kw + w],
                            start=st, stop=sp,
                            tile_position=(64, 0),
                        )
                # DVE drains both (ACT is ~2.6x slower at PSUM copies and is
                # kept free for the binarize sign ops)
                otT = opool.tile([64, R * w], F32, name="otT", tag="otT")
                nc.vector.tensor_scalar_mul(otT[:], psumT[:], sc[:])
                nc.sync.dma_start(out_ext[img, :, h0 : h0 + R, :], otT[:])
                otB = opool.tile([64, R * w], F32, name="otB", tag="otB")
                nc.vector.tensor_scalar_mul(otB[:], psumB[:], sc[:])
                nc.sync.dma_start(
                    out_ext[img, :, half + h0 : half + h0 + R, :], otB[:]
                )
    nc.finalize()
    return nc


_NC_CACHE = {}


def _get_nc():
    if "nc" not in _NC_CACHE:
        _NC_CACHE["nc"] = build_nc()
    return _NC_CACHE["nc"]


def _prep_weights(w):
    wc = np.clip(np.asarray(w, dtype=np.float32), -1.0, 1.0)
    scale = np.abs(wc).mean(axis=(1, 2, 3)).astype(np.float32).reshape(64, 1)
    s = np.sign(wc).astype(np.float32)  # [co, ci, kh, kw]
    wsgn = np.ascontiguousarray(
        np.transpose(s, (1, 2, 3, 0)).reshape(64, 9 * 64)
    )
    wsgn2 = np.concatenate([wsgn, wsgn], axis=0).astype(ml_dtypes.bfloat16)
    return wsgn2, scale


def kernel(x, w, _trace=False):
    x = np.ascontiguousarray(np.asarray(x, dtype=np.float32))
    wsgn2, scale = _prep_weights(w)
    nc = _get_nc()
    in_maps = [
        {"x": x[i * B_CORE : (i + 1) * B_CORE], "wsgn": wsgn2, "scale": scale}
        for i in range(N_CORES)
    ]
    # The axon-proxied execution occasionally faults with a transient
    # NRT_EXEC_UNIT_UNRECOVERABLE; a retry on a fresh session recovers.
    last_err = None
    for attempt in range(3):
        try:
            res = run_bass_kernel_spmd(nc, in_maps, list(range(N_CORES)), trace=_trace)
            break
        except Exception as e:  # noqa: BLE001
            last_err = e
            import time as _time
            _time.sleep(3.0)
    else:
        raise last_err
    out = np.concatenate([res.results[i]["out"] for i in range(N_CORES)], axis=0)
    if _trace:
        return out, res
    return out

